# revision 1
# baseline (speedup 1.0000x reference)
# Trainium2 Bass kernel for a 4-layer LSTM (B=32, T=2048, I=H=512),
# output = final cell states c_n (4, 32, 512).
#
# Strategy (mode "pipe", default):
#   8 cores = 4 layers x 2 batch halves. Each core runs ONE layer's
#   recurrence for a 16-sample batch half. Layer l+1 consumes layer l's
#   hidden-state sequence block-by-block (wavefront pipeline); blocks move
#   between cores with an AllGather over each 4-core chain per block.
# Mode "split" (fallback): batch-parallel only, each core runs all 4 layers
#   for 4 samples serially.
#
# Layout: everything gate-major. Weights pre-transposed/cast to bf16 on the
# host: lhsT tiles are (k_part=128, gate). Hidden state h kept as
# (128 h-dims, k-tile, batch) bf16 in SBUF; c state fp32. Input projections
# (xg) are precomputed per block with large efficient matmuls; the
# sequential recurrence streams Whh through the PE array each step
# (64 LDW+MM pairs, LDW-bound with automatic fast-weight-load for bf16).

import os
import numpy as np
import ml_dtypes

import concourse.bass as bass
import concourse.tile as tile
from concourse import bacc, mybir
from concourse.bass import ds
from concourse.bass_utils import run_bass_kernel_spmd
from concourse.expressions import smin, smax, s_not_equal

BF16 = mybir.dt.bfloat16
FP32 = mybir.dt.float32

# Problem constants (hardcoded per the contract)
B, T, I = 32, 2048, 512
H, L, G = 512, 4, 2048  # G = 4*H gates
KT = 4        # k tiles (512 / 128)
MT = 16       # m (gate) tiles (2048 / 128)
P = 128

MODE = os.environ.get("LSTM_MODE", "pipe")
T_OV = int(os.environ.get("LSTM_T_OVERRIDE", "0")) or T  # dev-only override
U_STEPS = int(os.environ.get("LSTM_U", "16"))
NO_CC = bool(int(os.environ.get("LSTM_NO_CC", "0")))  # timing diagnostic only
STAGGER = bool(int(os.environ.get("LSTM_STAGGER", "0")))
FAKE_STEPS = int(os.environ.get("LSTM_FAKE_STEPS", "-1"))  # diagnostic only
NO_PHA = bool(int(os.environ.get("LSTM_NO_PHA", "0")))  # diagnostic only
XG_DT_ENV = os.environ.get("LSTM_XG16", "0")  # bf16 xg scratch (halves DMA)

# gate-tile order in the fused weight layout: i,f,o,g so the three sigmoid
# regions are contiguous (2 ACT calls instead of 3)
GORDER = (0, 1, 3, 2)  # block g <-> o swap applied to (i,f,g,o) weight rows

_cache = {}


def _bf16(a):
    return np.asarray(a, np.float32).astype(ml_dtypes.bfloat16)


def _perm_gates(w):
    """Reorder gate blocks (i,f,g,o) -> (i,f,o,g) along dim 1 of (L, 4H, ...)."""
    w = np.asarray(w)
    blocks = w.reshape(w.shape[0], 4, H, *w.shape[2:])
    return np.ascontiguousarray(blocks[:, GORDER].reshape(w.shape))


# ---------------------------------------------------------------------------
# shared emitters
# ---------------------------------------------------------------------------

def _emit_phase_a(nc, pools, wih_sb, bias_sb, src_ap, src_roff, xg_dram, rows):
    """xg[g, r] = Wih.T @ inp + bias for `rows` rows starting at src_roff
    (ScalarValue or int offset into src_ap's last dim). Writes xg_dram
    (MT, 128, rows) fp32."""
    CH = 512
    nch = rows // CH
    for c in range(nch):
        inp = pools["mov"].tile([P, KT, CH], BF16, tag="mov")
        off = src_roff + c * CH if not isinstance(src_roff, int) else src_roff + c * CH
        nc.sync.dma_start(
            out=inp, in_=src_ap[:, :, ds(off, CH)].rearrange("a p c -> p a c")
        )
        for m in range(MT):
            ps = pools["psA"].tile([P, CH], FP32, tag="psA")
            for k in range(KT):
                nc.tensor.matmul(
                    ps,
                    lhsT=wih_sb[:, k, m * P:(m + 1) * P],
                    rhs=inp[:, k, :],
                    start=(k == 0),
                    stop=(k == KT - 1),
                )
            xs = pools["xgs"].tile([P, CH], xg_dram.dtype, tag="xgs")
            nc.vector.tensor_scalar_add(xs, ps, bias_sb[:, m:m + 1])
            nc.sync.dma_start(out=xg_dram[m, :, c * CH:(c + 1) * CH], in_=xs)


def _emit_steps(nc, tc, pools, whh_sb, xg_dram, h_sb, c_sb, hseq_ap, hseq_roff,
                nsteps, Bc, U, hint):
    """The sequential recurrence: nsteps LSTM steps. Reads xg_dram
    (MT,128,nsteps*Bc) fp32; h_sb (128,KT,Bc) bf16 and c_sb (128,KT,Bc) fp32
    updated in place; writes h rows into hseq_ap[:, :, hseq_roff + s]."""
    rows_per_iter = U * Bc

    with tc.For_i(0, nsteps * Bc, rows_per_iter, hint_engines=hint,
                  staggered_reset=STAGGER) as s:
        xg_u = pools["xgu"].tile([P, MT, rows_per_iter], xg_dram.dtype, tag="xgu")
        nc.sync.dma_start(
            out=xg_u,
            in_=xg_dram[:, :, ds(s, rows_per_iter)].rearrange("m p c -> p m c"),
        )
        hfl = pools["hfl"].tile([P, KT, rows_per_iter], BF16, tag="hfl")
        for u in range(U):
            ps = pools["psB"].tile([P, MT, Bc], FP32, tag="psB")
            for m in range(MT):
                for k in range(KT):
                    nc.tensor.matmul(
                        ps[:, m, :],
                        lhsT=whh_sb[:, k, m * P:(m + 1) * P],
                        rhs=h_sb[:, k, :],
                        start=(k == 0),
                        stop=(k == KT - 1),
                    )
            z = pools["z"].tile([P, MT, Bc], FP32, tag="z")
            nc.vector.tensor_add(out=z, in0=ps, in1=xg_u[:, :, u * Bc:(u + 1) * Bc])
            # weight rows are pre-permuted to (i, f, o, g) on the host
            gts = pools["g"].tile([P, MT, Bc], FP32, tag="g")
            nc.scalar.activation(gts[:, 0:12, :], z[:, 0:12, :],
                                 mybir.ActivationFunctionType.Sigmoid)
            nc.scalar.activation(gts[:, 12:16, :], z[:, 12:16, :],
                                 mybir.ActivationFunctionType.Tanh)
            t1 = pools["t1"].tile([P, KT, Bc], FP32, tag="t1")
            t2 = pools["t2"].tile([P, KT, Bc], FP32, tag="t2")
            nc.vector.tensor_mul(t1, gts[:, 4:8, :], c_sb)      # f * c
            nc.vector.tensor_mul(t2, gts[:, 0:4, :], gts[:, 12:16, :])  # i * g
            nc.vector.tensor_add(c_sb, t1, t2)
            tc_t = pools["tc"].tile([P, KT, Bc], FP32, tag="tc")
            nc.scalar.activation(tc_t, c_sb, mybir.ActivationFunctionType.Tanh)
            nc.vector.tensor_mul(h_sb, gts[:, 8:12, :], tc_t)  # o * tanh(c) -> bf16
            nc.vector.tensor_copy(out=hfl[:, :, u * Bc:(u + 1) * Bc], in_=h_sb)
        hout_off = hseq_roff + s if not isinstance(hseq_roff, int) else hseq_roff + s
        nc.sync.dma_start(
            out=hseq_ap[:, :, ds(hout_off, rows_per_iter)].rearrange("a p c -> p a c"),
            in_=hfl,
        )


def _make_pools(tc, ctx, Bc, U):
    pools = {}
    pools["mov"] = ctx.enter_context(tc.tile_pool(name="mov", bufs=3))
    pools["psA"] = ctx.enter_context(tc.tile_pool(name="psA", bufs=2, space="PSUM"))
    pools["xgs"] = ctx.enter_context(tc.tile_pool(name="xgs", bufs=3))
    pools["xgu"] = ctx.enter_context(tc.tile_pool(name="xgu", bufs=2))
    pools["hfl"] = ctx.enter_context(tc.tile_pool(name="hfl", bufs=2))
    pools["psB"] = ctx.enter_context(tc.tile_pool(name="psB", bufs=2, space="PSUM"))
    for nm in ("z", "g", "t1", "t2", "tc"):
        pools[nm] = ctx.enter_context(tc.tile_pool(name=nm, bufs=2))
    return pools


# ---------------------------------------------------------------------------
# mode "split": batch-parallel, all layers per core
# ---------------------------------------------------------------------------

def _build_split(Tl):
    Bc = B // 8  # 4
    U = U_STEPS
    RT = Tl * Bc
    nc = bacc.Bacc("TRN2", target_bir_lowering=False, debug=False, num_devices=8)
    xT = nc.dram_tensor("xT", [KT, P, RT], BF16, kind="ExternalInput").ap()
    wih = nc.dram_tensor("wihT", [L, KT, P, G], BF16, kind="ExternalInput").ap()
    whh = nc.dram_tensor("whhT", [L, KT, P, G], BF16, kind="ExternalInput").ap()
    bias = nc.dram_tensor("bias", [L, MT, P], FP32, kind="ExternalInput").ap()
    h0 = nc.dram_tensor("h0T", [L, KT, P, Bc], BF16, kind="ExternalInput").ap()
    c0 = nc.dram_tensor("c0T", [L, KT, P, Bc], FP32, kind="ExternalInput").ap()
    cout = nc.dram_tensor("cT", [L, KT, P, Bc], FP32, kind="ExternalOutput").ap()

    XG = BF16 if XG_DT_ENV == "1" else FP32
    xg_d = nc.dram_tensor("xg", [MT, P, RT], XG, kind="Internal").ap()
    hs_a = nc.dram_tensor("hseqA", [KT, P, RT], BF16, kind="Internal").ap()
    hs_b = nc.dram_tensor("hseqB", [KT, P, RT], BF16, kind="Internal").ap()

    from contextlib import ExitStack
    with tile.TileContext(nc) as tc, ExitStack() as ctx:
        pools = _make_pools(tc, ctx, Bc, U)
        singles = ctx.enter_context(tc.tile_pool(name="singles", bufs=1))
        wih_sb = singles.tile([P, KT, G], BF16, tag="wih")
        whh_sb = singles.tile([P, KT, G], BF16, tag="whh")
        bias_sb = singles.tile([P, MT], FP32, tag="bias")
        h_sb = singles.tile([P, KT, Bc], BF16, tag="h")
        c_sb = singles.tile([P, KT, Bc], FP32, tag="c")
        hint = (mybir.EngineType.PE, mybir.EngineType.DVE,
                mybir.EngineType.Activation, mybir.EngineType.SP)

        for l in range(L):
            nc.sync.dma_start(out=wih_sb, in_=wih[l].rearrange("a p g -> p a g"))
            nc.sync.dma_start(out=whh_sb, in_=whh[l].rearrange("a p g -> p a g"))
            nc.sync.dma_start(out=bias_sb, in_=bias[l].rearrange("m p -> p m"))
            nc.sync.dma_start(out=h_sb, in_=h0[l].rearrange("a p b -> p a b"))
            nc.sync.dma_start(out=c_sb, in_=c0[l].rearrange("a p b -> p a b"))
            src = xT if l == 0 else (hs_a if l % 2 == 1 else hs_b)
            dst = hs_a if l % 2 == 0 else hs_b
            _emit_phase_a(nc, pools, wih_sb, bias_sb, src, 0, xg_d, RT)
            _emit_steps(nc, tc, pools, whh_sb, xg_d, h_sb, c_sb, dst, 0,
                        Tl, Bc, U, hint)
            nc.sync.dma_start(out=cout[l].rearrange("a p b -> p a b"), in_=c_sb)
    nc.compile()
    return nc


def _prep_split(x, h0, c0, w_ih, w_hh, b_ih, b_hh, Tl):
    Bc = B // 8
    w_ih, w_hh = _perm_gates(w_ih), _perm_gates(w_hh)
    b_ih, b_hh = _perm_gates(b_ih[..., None])[..., 0], _perm_gates(b_hh[..., None])[..., 0]
    ins = []
    wihT = np.ascontiguousarray(
        _bf16(w_ih).transpose(0, 2, 1).reshape(L, KT, P, G))
    whhT = np.ascontiguousarray(
        _bf16(w_hh).transpose(0, 2, 1).reshape(L, KT, P, G))
    bias = np.ascontiguousarray(
        (np.asarray(b_ih, np.float32) + np.asarray(b_hh, np.float32))
        .reshape(L, MT, P))
    for c in range(8):
        bs = slice(c * Bc, (c + 1) * Bc)
        # xT[kt, p, t*Bc + b] = x[b, t, kt*128+p]
        xc = np.asarray(x[bs, :Tl, :], np.float32)  # (Bc, Tl, I)
        xT = np.ascontiguousarray(
            _bf16(xc).transpose(2, 1, 0).reshape(KT, P, Tl * Bc))
        h0T = np.ascontiguousarray(
            _bf16(h0[:, bs, :]).transpose(0, 2, 1).reshape(L, KT, P, Bc))
        c0T = np.ascontiguousarray(
            np.asarray(c0[:, bs, :], np.float32).transpose(0, 2, 1)
            .reshape(L, KT, P, Bc))
        ins.append({"xT": xT, "wihT": wihT, "whhT": whhT, "bias": bias,
                    "h0T": h0T, "c0T": c0T})
    return ins


def _post_split(results):
    Bc = B // 8
    out = np.zeros((L, B, H), np.float32)
    for c, r in enumerate(results):
        ct = r["cT"]  # (L, KT, P, Bc)
        out[:, c * Bc:(c + 1) * Bc, :] = ct.reshape(L, H, Bc).transpose(0, 2, 1)
    return out


# ---------------------------------------------------------------------------
# mode "pipe": layer pipeline x batch halves
# ---------------------------------------------------------------------------

def _build_pipe(Tl, BLK):
    Bc = B // 2  # 16
    U = U_STEPS
    NB = Tl // BLK
    RB = BLK * Bc          # rows per block
    RT = Tl * Bc
    LAG = L - 1
    nc = bacc.Bacc("TRN2", target_bir_lowering=False, debug=False, num_devices=8)
    xT = nc.dram_tensor("xT", [KT, P, RT], BF16, kind="ExternalInput").ap()
    wih = nc.dram_tensor("wihT", [KT, P, G], BF16, kind="ExternalInput").ap()
    whh = nc.dram_tensor("whhT", [KT, P, G], BF16, kind="ExternalInput").ap()
    bias = nc.dram_tensor("bias", [MT, P], FP32, kind="ExternalInput").ap()
    h0 = nc.dram_tensor("h0T", [KT, P, Bc], BF16, kind="ExternalInput").ap()
    c0 = nc.dram_tensor("c0T", [KT, P, Bc], FP32, kind="ExternalInput").ap()
    # ctrl scalars: [l, l*RB, prev_slot]
    ctrl = nc.dram_tensor("ctrl", [1, 4], mybir.dt.uint32, kind="ExternalInput").ap()
    cout = nc.dram_tensor("cT", [KT, P, Bc], FP32, kind="ExternalOutput").ap()

    XG = BF16 if XG_DT_ENV == "1" else FP32
    xg_d = nc.dram_tensor("xg", [MT, P, RB], XG, kind="Internal").ap()
    sendb = nc.dram_tensor("sendb", [KT, P, RB], BF16, kind="Internal").ap()
    gath = nc.dram_tensor("gath", [4, KT, P, RB], BF16, kind="Internal").ap()

    from contextlib import ExitStack
    with tile.TileContext(nc) as tc, ExitStack() as ctx:
        pools = _make_pools(tc, ctx, Bc, U)
        singles = ctx.enter_context(tc.tile_pool(name="singles", bufs=1))
        wih_sb = singles.tile([P, KT, G], BF16, tag="wih")
        whh_sb = singles.tile([P, KT, G], BF16, tag="whh")
        bias_sb = singles.tile([P, MT], FP32, tag="bias")
        h_sb = singles.tile([P, KT, Bc], BF16, tag="h")
        c_sb = singles.tile([P, KT, Bc], FP32, tag="c")
        hint = (mybir.EngineType.PE, mybir.EngineType.DVE,
                mybir.EngineType.Activation, mybir.EngineType.SP)

        nc.sync.dma_start(out=wih_sb, in_=wih.rearrange("a p g -> p a g"))
        nc.sync.dma_start(out=whh_sb, in_=whh.rearrange("a p g -> p a g"))
        nc.sync.dma_start(out=bias_sb, in_=bias.rearrange("m p -> p m"))

        eng = nc.sync
        l_sv = _load_ctrl(nc, eng, ctrl, 0, 3)
        lrb_sv = _load_ctrl(nc, eng, ctrl, 1, LAG * RB)
        pslot_sv = _load_ctrl(nc, eng, ctrl, 2, 3)

        for j in range(NB + LAG):
            # block index this core works on: clamp(j - l, 0, NB-1) * RB
            roff = smax(smin(j * RB - lrb_sv, (NB - 1) * RB), 0)
            # exchange h blocks (contents of sendb were written in iter j-1)
            if not NO_CC:
                nc.gpsimd.collective_compute(
                    kind="AllGather", op=mybir.AluOpType.bypass,
                    replica_groups=[[0, 1, 2, 3], [4, 5, 6, 7]],
                    ins=[sendb], outs=[gath],
                )
            # receive predecessor's block into my input sequence (l>0 only)
            nc.sync.dma_start(
                out=xT[:, :, ds(roff, RB)],
                in_=gath[ds(pslot_sv, 1), :, :, :].rearrange("o a p c -> (o a) p c"),
                cond=s_not_equal(l_sv, 0),
            )
            # state init on my first real block
            is_first = 1 - s_not_equal(l_sv, j)
            nc.sync.dma_start(out=h_sb, in_=h0.rearrange("a p b -> p a b"),
                              cond=is_first)
            nc.sync.dma_start(out=c_sb, in_=c0.rearrange("a p b -> p a b"),
                              cond=is_first)
            if not NO_PHA:
                _emit_phase_a(nc, pools, wih_sb, bias_sb, xT, roff, xg_d, RB)
            nst = BLK if FAKE_STEPS < 0 else FAKE_STEPS
            if nst:
                _emit_steps(nc, tc, pools, whh_sb, xg_d, h_sb, c_sb, sendb, 0,
                            nst, Bc, U, hint)
            # write final c on my last real block
            is_last = 1 - s_not_equal(l_sv, j - NB + 1)
            nc.sync.dma_start(out=cout.rearrange("a p b -> p a b"), in_=c_sb,
                              cond=is_last)
    nc.compile()
    return nc


def _load_ctrl(nc, eng, ctrl, idx, max_val):
    reg = eng.alloc_register(f"ctrl{idx}")
    eng.reg_load(reg, ctrl[0:1, idx:idx + 1])
    return eng.snap(reg, donate=True, min_val=0, max_val=max_val)


def _prep_pipe(x, h0, c0, w_ih, w_hh, b_ih, b_hh, Tl, BLK):
    Bc = B // 2
    w_ih, w_hh = _perm_gates(w_ih), _perm_gates(w_hh)
    b_ih, b_hh = _perm_gates(b_ih[..., None])[..., 0], _perm_gates(b_hh[..., None])[..., 0]
    RB = BLK * Bc
    bias_all = (np.asarray(b_ih, np.float32) + np.asarray(b_hh, np.float32))
    wihT = np.ascontiguousarray(_bf16(w_ih).transpose(0, 2, 1).reshape(L, KT, P, G))
    whhT = np.ascontiguousarray(_bf16(w_hh).transpose(0, 2, 1).reshape(L, KT, P, G))
    ins = []
    for c in range(8):
        half, l = c // 4, c % 4
        bs = slice(half * Bc, (half + 1) * Bc)
        xc = np.asarray(x[bs, :Tl, :], np.float32)
        xT = np.ascontiguousarray(_bf16(xc).transpose(2, 1, 0).reshape(KT, P, Tl * Bc))
        h0T = np.ascontiguousarray(_bf16(h0[l, bs, :]).T.reshape(KT, P, Bc))
        c0T = np.ascontiguousarray(
            np.asarray(c0[l, bs, :], np.float32).T.reshape(KT, P, Bc))
        ctrl = np.array([[l, l * RB, (l + 3) % 4, 0]], np.uint32)
        ins.append({"xT": xT, "wihT": wihT[l], "whhT": whhT[l],
                    "bias": bias_all[l].reshape(MT, P), "h0T": h0T, "c0T": c0T,
                    "ctrl": ctrl})
    return ins


def _post_pipe(results):
    Bc = B // 2
    out = np.zeros((L, B, H), np.float32)
    for c, r in enumerate(results):
        half, l = c // 4, c % 4
        ct = r["cT"]  # (KT, P, Bc)
        out[l, half * Bc:(half + 1) * Bc, :] = ct.reshape(H, Bc).T
    return out


# ---------------------------------------------------------------------------

def _get_built(mode, Tl):
    key = (mode, Tl)
    if key not in _cache:
        if mode == "split":
            _cache[key] = _build_split(Tl)
        else:
            BLK = int(os.environ.get("LSTM_BLK", "256"))
            _cache[key] = _build_pipe(Tl, BLK)
    return _cache[key]


def kernel(x, h0, c0, w_ih, w_hh, b_ih, b_hh):
    Tl = min(T_OV, np.asarray(x).shape[1])
    nc = _get_built(MODE, Tl)
    if MODE == "split":
        ins = _prep_split(x, h0, c0, w_ih, w_hh, b_ih, b_hh, Tl)
    else:
        BLK = int(os.environ.get("LSTM_BLK", "256"))
        ins = _prep_pipe(x, h0, c0, w_ih, w_hh, b_ih, b_hh, Tl, BLK)
    res = run_bass_kernel_spmd(nc, ins, core_ids=list(range(8)))
    out = _post_split(res.results) if MODE == "split" else _post_pipe(res.results)
    return out



# revision 13
# speedup vs baseline: 1.4593x; 1.4593x over previous
# Trainium2 Bass kernel for a 4-layer LSTM (B=32, T=2048, I=H=512),
# output = final cell states c_n (4, 32, 512).
#
# Strategy (mode "pipe", default):
#   8 cores = 4 layers x 2 batch halves. Each core runs ONE layer's
#   recurrence for a 16-sample batch half. Layer l+1 consumes layer l's
#   hidden-state sequence block-by-block (wavefront pipeline); blocks move
#   between cores with an AllGather over each 4-core chain per block.
# Mode "split" (fallback): batch-parallel only, each core runs all 4 layers
#   for 4 samples serially.
#
# Layout: everything gate-major. Weights pre-transposed/cast to bf16 on the
# host: lhsT tiles are (k_part=128, gate). Hidden state h kept as
# (128 h-dims, k-tile, batch) bf16 in SBUF; c state fp32. Input projections
# (xg) are precomputed per block with large efficient matmuls; the
# sequential recurrence streams Whh through the PE array each step
# (64 LDW+MM pairs, LDW-bound with automatic fast-weight-load for bf16).

import os
import numpy as np
import ml_dtypes

import concourse.bass as bass
import concourse.tile as tile
from concourse import bacc, mybir
from concourse.bass import ds
from concourse.bass_utils import run_bass_kernel_spmd
from concourse.expressions import smin, smax, s_not_equal

BF16 = mybir.dt.bfloat16
FP32 = mybir.dt.float32

# Problem constants (hardcoded per the contract)
B, T, I = 32, 2048, 512
H, L, G = 512, 4, 2048  # G = 4*H gates
KT = 4        # k tiles (512 / 128)
MT = 16       # m (gate) tiles (2048 / 128)
P = 128

MODE = os.environ.get("LSTM_MODE", "pipe2")
BC2 = 16      # per-core batch in pipe2 (half of B)
T_OV = int(os.environ.get("LSTM_T_OVERRIDE", "0")) or T  # dev-only override
U_STEPS = int(os.environ.get("LSTM_U", "16"))
NO_CC = bool(int(os.environ.get("LSTM_NO_CC", "0")))  # timing diagnostic only
STAGGER = bool(int(os.environ.get("LSTM_STAGGER", "0")))
FAKE_STEPS = int(os.environ.get("LSTM_FAKE_STEPS", "-1"))  # diagnostic only
NO_PHA = bool(int(os.environ.get("LSTM_NO_PHA", "0")))  # diagnostic only
XG_DT_ENV = os.environ.get("LSTM_XG16", "0")  # bf16 xg scratch (halves DMA)
XG2 = mybir.dt.float32 if os.environ.get("LSTM_XG32", "0") == "1" else mybir.dt.bfloat16
KINNER = bool(int(os.environ.get("LSTM_KINNER", "0")))

# gate-tile order in the fused weight layout: i,f,o,g so the three sigmoid
# regions are contiguous (2 ACT calls instead of 3)
GORDER = (0, 1, 3, 2)  # block g <-> o swap applied to (i,f,g,o) weight rows

_cache = {}


def _bf16(a):
    return np.asarray(a, np.float32).astype(ml_dtypes.bfloat16)


def _perm_gates(w):
    """Reorder gate blocks (i,f,g,o) -> (i,f,o,g) along dim 1 of (L, 4H, ...)."""
    w = np.asarray(w)
    blocks = w.reshape(w.shape[0], 4, H, *w.shape[2:])
    return np.ascontiguousarray(blocks[:, GORDER].reshape(w.shape))


# ---------------------------------------------------------------------------
# shared emitters
# ---------------------------------------------------------------------------

def _emit_phase_a(nc, pools, wih_sb, bias_sb, src_ap, src_roff, xg_dram, rows):
    """xg[g, r] = Wih.T @ inp + bias for `rows` rows starting at src_roff
    (ScalarValue or int offset into src_ap's last dim). Writes xg_dram
    (MT, 128, rows) fp32."""
    CH = 512
    nch = rows // CH
    for c in range(nch):
        inp = pools["mov"].tile([P, KT, CH], BF16, tag="mov")
        off = src_roff + c * CH if not isinstance(src_roff, int) else src_roff + c * CH
        nc.sync.dma_start(
            out=inp, in_=src_ap[:, :, ds(off, CH)].rearrange("a p c -> p a c")
        )
        for m in range(MT):
            ps = pools["psA"].tile([P, CH], FP32, tag="psA")
            for k in range(KT):
                nc.tensor.matmul(
                    ps,
                    lhsT=wih_sb[:, k, m * P:(m + 1) * P],
                    rhs=inp[:, k, :],
                    start=(k == 0),
                    stop=(k == KT - 1),
                )
            xs = pools["xgs"].tile([P, CH], xg_dram.dtype, tag="xgs")
            nc.vector.tensor_scalar_add(xs, ps, bias_sb[:, m:m + 1])
            nc.sync.dma_start(out=xg_dram[m, :, c * CH:(c + 1) * CH], in_=xs)


def _emit_steps(nc, tc, pools, whh_sb, xg_dram, h_sb, c_sb, hseq_ap, hseq_roff,
                nsteps, Bc, U, hint):
    """The sequential recurrence: nsteps LSTM steps. Reads xg_dram
    (MT,128,nsteps*Bc) fp32; h_sb (128,KT,Bc) bf16 and c_sb (128,KT,Bc) fp32
    updated in place; writes h rows into hseq_ap[:, :, hseq_roff + s]."""
    rows_per_iter = U * Bc

    with tc.For_i(0, nsteps * Bc, rows_per_iter, hint_engines=hint,
                  staggered_reset=STAGGER) as s:
        xg_u = pools["xgu"].tile([P, MT, rows_per_iter], xg_dram.dtype, tag="xgu")
        nc.sync.dma_start(
            out=xg_u,
            in_=xg_dram[:, :, ds(s, rows_per_iter)].rearrange("m p c -> p m c"),
        )
        hfl = pools["hfl"].tile([P, KT, rows_per_iter], BF16, tag="hfl")
        for u in range(U):
            ps = pools["psB"].tile([P, MT, Bc], FP32, tag="psB")
            for m in range(MT):
                for k in range(KT):
                    nc.tensor.matmul(
                        ps[:, m, :],
                        lhsT=whh_sb[:, k, m * P:(m + 1) * P],
                        rhs=h_sb[:, k, :],
                        start=(k == 0),
                        stop=(k == KT - 1),
                    )
            z = pools["z"].tile([P, MT, Bc], FP32, tag="z")
            nc.vector.tensor_add(out=z, in0=ps, in1=xg_u[:, :, u * Bc:(u + 1) * Bc])
            # weight rows are pre-permuted to (i, f, o, g) on the host
            gts = pools["g"].tile([P, MT, Bc], FP32, tag="g")
            nc.scalar.activation(gts[:, 0:12, :], z[:, 0:12, :],
                                 mybir.ActivationFunctionType.Sigmoid)
            nc.scalar.activation(gts[:, 12:16, :], z[:, 12:16, :],
                                 mybir.ActivationFunctionType.Tanh)
            t1 = pools["t1"].tile([P, KT, Bc], FP32, tag="t1")
            t2 = pools["t2"].tile([P, KT, Bc], FP32, tag="t2")
            nc.vector.tensor_mul(t1, gts[:, 4:8, :], c_sb)      # f * c
            nc.vector.tensor_mul(t2, gts[:, 0:4, :], gts[:, 12:16, :])  # i * g
            nc.vector.tensor_add(c_sb, t1, t2)
            tc_t = pools["tc"].tile([P, KT, Bc], FP32, tag="tc")
            nc.scalar.activation(tc_t, c_sb, mybir.ActivationFunctionType.Tanh)
            nc.vector.tensor_mul(h_sb, gts[:, 8:12, :], tc_t)  # o * tanh(c) -> bf16
            nc.vector.tensor_copy(out=hfl[:, :, u * Bc:(u + 1) * Bc], in_=h_sb)
        hout_off = hseq_roff + s if not isinstance(hseq_roff, int) else hseq_roff + s
        nc.sync.dma_start(
            out=hseq_ap[:, :, ds(hout_off, rows_per_iter)].rearrange("a p c -> p a c"),
            in_=hfl,
        )


def _make_pools(tc, ctx, Bc, U):
    pools = {}
    pools["mov"] = ctx.enter_context(tc.tile_pool(name="mov", bufs=3))
    pools["psA"] = ctx.enter_context(tc.tile_pool(name="psA", bufs=2, space="PSUM"))
    pools["xgs"] = ctx.enter_context(tc.tile_pool(name="xgs", bufs=3))
    pools["xgu"] = ctx.enter_context(tc.tile_pool(name="xgu", bufs=2))
    pools["hfl"] = ctx.enter_context(tc.tile_pool(name="hfl", bufs=2))
    pools["psB"] = ctx.enter_context(tc.tile_pool(name="psB", bufs=2, space="PSUM"))
    for nm in ("z", "g", "t1", "t2", "tc"):
        pools[nm] = ctx.enter_context(tc.tile_pool(name=nm, bufs=2))
    return pools


# ---------------------------------------------------------------------------
# mode "split": batch-parallel, all layers per core
# ---------------------------------------------------------------------------

def _build_split(Tl):
    Bc = B // 8  # 4
    U = U_STEPS
    RT = Tl * Bc
    nc = bacc.Bacc("TRN2", target_bir_lowering=False, debug=False, num_devices=8)
    xT = nc.dram_tensor("xT", [KT, P, RT], BF16, kind="ExternalInput").ap()
    wih = nc.dram_tensor("wihT", [L, KT, P, G], BF16, kind="ExternalInput").ap()
    whh = nc.dram_tensor("whhT", [L, KT, P, G], BF16, kind="ExternalInput").ap()
    bias = nc.dram_tensor("bias", [L, MT, P], FP32, kind="ExternalInput").ap()
    h0 = nc.dram_tensor("h0T", [L, KT, P, Bc], BF16, kind="ExternalInput").ap()
    c0 = nc.dram_tensor("c0T", [L, KT, P, Bc], FP32, kind="ExternalInput").ap()
    cout = nc.dram_tensor("cT", [L, KT, P, Bc], FP32, kind="ExternalOutput").ap()

    XG = BF16 if XG_DT_ENV == "1" else FP32
    xg_d = nc.dram_tensor("xg", [MT, P, RT], XG, kind="Internal").ap()
    hs_a = nc.dram_tensor("hseqA", [KT, P, RT], BF16, kind="Internal").ap()
    hs_b = nc.dram_tensor("hseqB", [KT, P, RT], BF16, kind="Internal").ap()

    from contextlib import ExitStack
    with tile.TileContext(nc) as tc, ExitStack() as ctx:
        pools = _make_pools(tc, ctx, Bc, U)
        singles = ctx.enter_context(tc.tile_pool(name="singles", bufs=1))
        wih_sb = singles.tile([P, KT, G], BF16, tag="wih")
        whh_sb = singles.tile([P, KT, G], BF16, tag="whh")
        bias_sb = singles.tile([P, MT], FP32, tag="bias")
        h_sb = singles.tile([P, KT, Bc], BF16, tag="h")
        c_sb = singles.tile([P, KT, Bc], FP32, tag="c")
        hint = (mybir.EngineType.PE, mybir.EngineType.DVE,
                mybir.EngineType.Activation, mybir.EngineType.SP)

        for l in range(L):
            nc.sync.dma_start(out=wih_sb, in_=wih[l].rearrange("a p g -> p a g"))
            nc.sync.dma_start(out=whh_sb, in_=whh[l].rearrange("a p g -> p a g"))
            nc.sync.dma_start(out=bias_sb, in_=bias[l].rearrange("m p -> p m"))
            nc.sync.dma_start(out=h_sb, in_=h0[l].rearrange("a p b -> p a b"))
            nc.sync.dma_start(out=c_sb, in_=c0[l].rearrange("a p b -> p a b"))
            src = xT if l == 0 else (hs_a if l % 2 == 1 else hs_b)
            dst = hs_a if l % 2 == 0 else hs_b
            _emit_phase_a(nc, pools, wih_sb, bias_sb, src, 0, xg_d, RT)
            _emit_steps(nc, tc, pools, whh_sb, xg_d, h_sb, c_sb, dst, 0,
                        Tl, Bc, U, hint)
            nc.sync.dma_start(out=cout[l].rearrange("a p b -> p a b"), in_=c_sb)
    nc.compile()
    return nc


def _prep_split(x, h0, c0, w_ih, w_hh, b_ih, b_hh, Tl):
    Bc = B // 8
    w_ih, w_hh = _perm_gates(w_ih), _perm_gates(w_hh)
    b_ih, b_hh = _perm_gates(b_ih[..., None])[..., 0], _perm_gates(b_hh[..., None])[..., 0]
    ins = []
    wihT = np.ascontiguousarray(
        _bf16(w_ih).transpose(0, 2, 1).reshape(L, KT, P, G))
    whhT = np.ascontiguousarray(
        _bf16(w_hh).transpose(0, 2, 1).reshape(L, KT, P, G))
    bias = np.ascontiguousarray(
        (np.asarray(b_ih, np.float32) + np.asarray(b_hh, np.float32))
        .reshape(L, MT, P))
    for c in range(8):
        bs = slice(c * Bc, (c + 1) * Bc)
        # xT[kt, p, t*Bc + b] = x[b, t, kt*128+p]
        xc = np.asarray(x[bs, :Tl, :], np.float32)  # (Bc, Tl, I)
        xT = np.ascontiguousarray(
            _bf16(xc).transpose(2, 1, 0).reshape(KT, P, Tl * Bc))
        h0T = np.ascontiguousarray(
            _bf16(h0[:, bs, :]).transpose(0, 2, 1).reshape(L, KT, P, Bc))
        c0T = np.ascontiguousarray(
            np.asarray(c0[:, bs, :], np.float32).transpose(0, 2, 1)
            .reshape(L, KT, P, Bc))
        ins.append({"xT": xT, "wihT": wihT, "whhT": whhT, "bias": bias,
                    "h0T": h0T, "c0T": c0T})
    return ins


def _post_split(results):
    Bc = B // 8
    out = np.zeros((L, B, H), np.float32)
    for c, r in enumerate(results):
        ct = r["cT"]  # (L, KT, P, Bc)
        out[:, c * Bc:(c + 1) * Bc, :] = ct.reshape(L, H, Bc).transpose(0, 2, 1)
    return out


# ---------------------------------------------------------------------------
# mode "pipe": layer pipeline x batch halves
# ---------------------------------------------------------------------------

def _build_pipe(Tl, BLK):
    Bc = B // 2  # 16
    U = U_STEPS
    NB = Tl // BLK
    RB = BLK * Bc          # rows per block
    RT = Tl * Bc
    LAG = L - 1
    nc = bacc.Bacc("TRN2", target_bir_lowering=False, debug=False, num_devices=8)
    xT = nc.dram_tensor("xT", [KT, P, RT], BF16, kind="ExternalInput").ap()
    wih = nc.dram_tensor("wihT", [KT, P, G], BF16, kind="ExternalInput").ap()
    whh = nc.dram_tensor("whhT", [KT, P, G], BF16, kind="ExternalInput").ap()
    bias = nc.dram_tensor("bias", [MT, P], FP32, kind="ExternalInput").ap()
    h0 = nc.dram_tensor("h0T", [KT, P, Bc], BF16, kind="ExternalInput").ap()
    c0 = nc.dram_tensor("c0T", [KT, P, Bc], FP32, kind="ExternalInput").ap()
    # ctrl scalars: [l, l*RB, prev_slot]
    ctrl = nc.dram_tensor("ctrl", [1, 4], mybir.dt.uint32, kind="ExternalInput").ap()
    cout = nc.dram_tensor("cT", [KT, P, Bc], FP32, kind="ExternalOutput").ap()

    XG = BF16 if XG_DT_ENV == "1" else FP32
    xg_d = nc.dram_tensor("xg", [MT, P, RB], XG, kind="Internal").ap()
    sendb = nc.dram_tensor("sendb", [KT, P, RB], BF16, kind="Internal").ap()
    gath = nc.dram_tensor("gath", [4, KT, P, RB], BF16, kind="Internal").ap()

    from contextlib import ExitStack
    with tile.TileContext(nc) as tc, ExitStack() as ctx:
        pools = _make_pools(tc, ctx, Bc, U)
        singles = ctx.enter_context(tc.tile_pool(name="singles", bufs=1))
        wih_sb = singles.tile([P, KT, G], BF16, tag="wih")
        whh_sb = singles.tile([P, KT, G], BF16, tag="whh")
        bias_sb = singles.tile([P, MT], FP32, tag="bias")
        h_sb = singles.tile([P, KT, Bc], BF16, tag="h")
        c_sb = singles.tile([P, KT, Bc], FP32, tag="c")
        hint = (mybir.EngineType.PE, mybir.EngineType.DVE,
                mybir.EngineType.Activation, mybir.EngineType.SP)

        nc.sync.dma_start(out=wih_sb, in_=wih.rearrange("a p g -> p a g"))
        nc.sync.dma_start(out=whh_sb, in_=whh.rearrange("a p g -> p a g"))
        nc.sync.dma_start(out=bias_sb, in_=bias.rearrange("m p -> p m"))

        eng = nc.sync
        l_sv = _load_ctrl(nc, eng, ctrl, 0, 3)
        lrb_sv = _load_ctrl(nc, eng, ctrl, 1, LAG * RB)
        pslot_sv = _load_ctrl(nc, eng, ctrl, 2, 3)

        for j in range(NB + LAG):
            # block index this core works on: clamp(j - l, 0, NB-1) * RB
            roff = smax(smin(j * RB - lrb_sv, (NB - 1) * RB), 0)
            # exchange h blocks (contents of sendb were written in iter j-1)
            if not NO_CC:
                nc.gpsimd.collective_compute(
                    kind="AllGather", op=mybir.AluOpType.bypass,
                    replica_groups=[[0, 1, 2, 3], [4, 5, 6, 7]],
                    ins=[sendb], outs=[gath],
                )
            # receive predecessor's block into my input sequence (l>0 only)
            nc.sync.dma_start(
                out=xT[:, :, ds(roff, RB)],
                in_=gath[ds(pslot_sv, 1), :, :, :].rearrange("o a p c -> (o a) p c"),
                cond=s_not_equal(l_sv, 0),
            )
            # state init on my first real block
            is_first = 1 - s_not_equal(l_sv, j)
            nc.sync.dma_start(out=h_sb, in_=h0.rearrange("a p b -> p a b"),
                              cond=is_first)
            nc.sync.dma_start(out=c_sb, in_=c0.rearrange("a p b -> p a b"),
                              cond=is_first)
            if not NO_PHA:
                _emit_phase_a(nc, pools, wih_sb, bias_sb, xT, roff, xg_d, RB)
            nst = BLK if FAKE_STEPS < 0 else FAKE_STEPS
            if nst:
                _emit_steps(nc, tc, pools, whh_sb, xg_d, h_sb, c_sb, sendb, 0,
                            nst, Bc, U, hint)
            # write final c on my last real block
            is_last = 1 - s_not_equal(l_sv, j - NB + 1)
            nc.sync.dma_start(out=cout.rearrange("a p b -> p a b"), in_=c_sb,
                              cond=is_last)
    nc.compile()
    return nc


def _load_ctrl(nc, eng, ctrl, idx, max_val):
    reg = eng.alloc_register(f"ctrl{idx}")
    eng.reg_load(reg, ctrl[0:1, idx:idx + 1])
    return eng.snap(reg, donate=True, min_val=0, max_val=max_val)


def _prep_pipe(x, h0, c0, w_ih, w_hh, b_ih, b_hh, Tl, BLK):
    Bc = B // 2
    w_ih, w_hh = _perm_gates(w_ih), _perm_gates(w_hh)
    b_ih, b_hh = _perm_gates(b_ih[..., None])[..., 0], _perm_gates(b_hh[..., None])[..., 0]
    RB = BLK * Bc
    bias_all = (np.asarray(b_ih, np.float32) + np.asarray(b_hh, np.float32))
    wihT = np.ascontiguousarray(_bf16(w_ih).transpose(0, 2, 1).reshape(L, KT, P, G))
    whhT = np.ascontiguousarray(_bf16(w_hh).transpose(0, 2, 1).reshape(L, KT, P, G))
    ins = []
    for c in range(8):
        half, l = c // 4, c % 4
        bs = slice(half * Bc, (half + 1) * Bc)
        xc = np.asarray(x[bs, :Tl, :], np.float32)
        xT = np.ascontiguousarray(_bf16(xc).transpose(2, 1, 0).reshape(KT, P, Tl * Bc))
        h0T = np.ascontiguousarray(_bf16(h0[l, bs, :]).T.reshape(KT, P, Bc))
        c0T = np.ascontiguousarray(
            np.asarray(c0[l, bs, :], np.float32).T.reshape(KT, P, Bc))
        ctrl = np.array([[l, l * RB, (l + 3) % 4, 0]], np.uint32)
        ins.append({"xT": xT, "wihT": wihT[l], "whhT": whhT[l],
                    "bias": bias_all[l].reshape(MT, P), "h0T": h0T, "c0T": c0T,
                    "ctrl": ctrl})
    return ins


def _post_pipe(results):
    Bc = B // 2
    out = np.zeros((L, B, H), np.float32)
    for c, r in enumerate(results):
        half, l = c // 4, c % 4
        ct = r["cT"]  # (KT, P, Bc)
        out[l, half * Bc:(half + 1) * Bc, :] = ct.reshape(H, Bc).T
    return out


# ---------------------------------------------------------------------------
# mode "pipe2": layer pipeline x batch halves, d=2 overlapped CC/phaseA,
# SBUF-resident xg, k-sweep step order with per-kappa-group chains.
# ---------------------------------------------------------------------------
# Gate-tile order m' = kappa*4 + gate with gate order (f, i, o, g); kappa is
# the 128-dim output h-group. Host permutes weight rows accordingly.

GORDER2 = (1, 0, 3, 2)  # per-group gate source blocks: f, i, o, g


def _perm_gates2(w):
    """Rows (4H) in blocks (i,f,g,o) -> m' order: for kappa, (f,i,o,g)[kappa]."""
    w = np.asarray(w)
    blocks = w.reshape(w.shape[0], 4, KT, P, *w.shape[2:])  # (L, gate, kappa, p, ...)
    out = blocks[:, GORDER2]  # (L, gate'(f,i,o,g), kappa, p, ...)
    out = out.transpose(0, 2, 1, *range(3, out.ndim))  # (L, kappa, gate', p, ...)
    return np.ascontiguousarray(out.reshape(w.shape))


def _emit_phase_a2(nc, pools, wih_sb, bias_sb, xg_dst, src_gath, pslot_sv, xT,
                   l0_roff, is_l0, rows, CH):
    """Compute xg[m', r] for `rows` rows into xg_dst (SBUF [P, MT, rows] bf16).
    Source: xT[:, :, l0_roff + ...] when is_l0 else gath[pslot]."""
    nch = rows // CH
    for c in range(nch):
        inp = pools["mov"].tile([P, KT, CH], BF16, tag="mov")
        nc.sync.dma_start(
            out=inp, in_=xT[:, :, ds(l0_roff + c * CH, CH)].rearrange("a p c -> p a c"),
            cond=is_l0,
        )
        nc.sync.dma_start(
            out=inp,
            in_=src_gath[ds(pslot_sv, 1), :, :, ds(c * CH, CH)]
            .rearrange("o a p c -> p (o a) c"),
            cond=1 - is_l0,
        )
        for m in range(MT):
            ps = pools["psA"].tile([P, CH], FP32, tag="psA")
            for k in range(KT):
                nc.tensor.matmul(
                    ps,
                    lhsT=wih_sb[:, k, m * P:(m + 1) * P],
                    rhs=inp[:, k, :],
                    start=(k == 0),
                    stop=(k == KT - 1),
                )
            nc.vector.tensor_scalar_add(
                xg_dst[:, m, c * CH:(c + 1) * CH], ps, bias_sb[:, m:m + 1])


def _emit_steps2(nc, tc, pools, whh_sb, xg_sb, h_k, cg_k, sendb, nsteps, U, hint):
    """nsteps LSTM steps; xg_sb SBUF [P, MT, nsteps*Bc]; h_k[kt] [P,Bc] bf16
    persistent; cg_k[kt] [P,2,Bc] fp32 persistent (c in [:,0,:]); h rows
    written to sendb DRAM [KT, P, nsteps*Bc]."""
    Bc = BC2
    rows_per_iter = U * Bc
    with tc.For_i(0, nsteps * Bc, rows_per_iter, hint_engines=hint,
                  staggered_reset=STAGGER) as s:
        xg_u = pools["xgu"].tile([P, MT, rows_per_iter], XG2, tag="xgu")
        nc.sync.dma_start(out=xg_u, in_=xg_sb[:, :, ds(s, rows_per_iter)])
        hfl = pools["hfl"].tile([P, KT, rows_per_iter], BF16, tag="hfl")
        for u in range(U):
            ps_g = [pools["psB"].tile([P, 4, Bc], FP32, tag="ps", name=f"ps{i}")
                    for i in range(4)]
            # k-input-ordered sweeps; within a sweep, m' ordered by kappa
            def rhs_k(k):
                return h_k[k] if u == 0 else hfl[:, k, (u - 1) * Bc:u * Bc]
            if KINNER:
                order = [(k, kap, g) for kap in range(4) for g in range(4)
                         for k in range(KT)]
            else:
                order = [(k, kap, g) for k in range(KT) for kap in range(4)
                         for g in range(4)]
            for (k, kap, g) in order:
                mp = kap * 4 + g
                # start=True clears has_written for the WHOLE bank; with
                # k-outer order only the bank's first matmul may set it.
                st = (k == 0) if KINNER else (k == 0 and g == 0)
                nc.tensor.matmul(
                    ps_g[kap][:, g, :],
                    lhsT=whh_sb[:, k, mp * P:(mp + 1) * P],
                    rhs=rhs_k(k),
                    start=st,
                    stop=(k == KT - 1),
                )
            for kap in range(4):
                ps = ps_g[kap]
                # z = psum + xg (in place, PSUM)
                nc.vector.tensor_add(
                    out=ps, in0=ps,
                    in1=xg_u[:, kap * 4:(kap + 1) * 4, u * Bc:(u + 1) * Bc])
                gts = pools["g"].tile([P, 3, Bc], FP32, tag="g")
                nc.scalar.activation(gts, ps[:, 0:3, :],
                                     mybir.ActivationFunctionType.Sigmoid)
                nc.scalar.activation(cg_k[kap][:, 1:2, :], ps[:, 3:4, :],
                                     mybir.ActivationFunctionType.Tanh)
                t12 = pools["t1"].tile([P, 2, Bc], FP32, tag="t1")
                nc.vector.tensor_mul(t12, gts[:, 0:2, :], cg_k[kap])  # f*c, i*tg
                nc.vector.tensor_add(cg_k[kap][:, 0:1, :],
                                     t12[:, 0:1, :], t12[:, 1:2, :])
                tcx = pools["tc"].tile([P, 1, Bc], FP32, tag="tc")
                nc.scalar.activation(tcx, cg_k[kap][:, 0:1, :],
                                     mybir.ActivationFunctionType.Tanh)
                nc.vector.tensor_mul(hfl[:, kap, u * Bc:(u + 1) * Bc],
                                     gts[:, 2, :], tcx[:, 0, :])
            if u == U - 1:
                for kap in range(4):
                    nc.scalar.copy(h_k[kap], hfl[:, kap, u * Bc:(u + 1) * Bc])
        nc.sync.dma_start(
            out=sendb[:, :, ds(s, rows_per_iter)].rearrange("a p c -> p a c"),
            in_=hfl,
        )


def _build_pipe2(Tl, BLK):
    Bc = BC2
    U = U_STEPS
    NB = Tl // BLK
    RB = BLK * Bc
    RT = Tl * Bc
    NJ = NB + 2 * (L - 1)
    CH = min(512, RB)
    nc = bacc.Bacc("TRN2", target_bir_lowering=False, debug=False, num_devices=8)
    xT = nc.dram_tensor("xT", [KT, P, RT], BF16, kind="ExternalInput").ap()
    wih = nc.dram_tensor("wihT", [KT, P, G], BF16, kind="ExternalInput").ap()
    whh = nc.dram_tensor("whhT", [KT, P, G], BF16, kind="ExternalInput").ap()
    bias = nc.dram_tensor("bias", [MT, P], FP32, kind="ExternalInput").ap()
    h0 = nc.dram_tensor("h0T", [KT, P, Bc], BF16, kind="ExternalInput").ap()
    c0 = nc.dram_tensor("c0T", [KT, P, Bc], FP32, kind="ExternalInput").ap()
    # ctrl scalars: [l, first_iter, last_iter, pslot]
    ctrl = nc.dram_tensor("ctrl", [1, 4], mybir.dt.uint32, kind="ExternalInput").ap()
    cout = nc.dram_tensor("cT", [KT, P, Bc], FP32, kind="ExternalOutput").ap()

    sendb = [nc.dram_tensor(f"sendb{i}", [KT, P, RB], BF16, kind="Internal").ap()
             for i in range(2)]
    gath = [nc.dram_tensor(f"gath{i}", [4, KT, P, RB], BF16, kind="Internal").ap()
            for i in range(2)]

    from contextlib import ExitStack
    with tile.TileContext(nc) as tc, ExitStack() as ctx:
        pools = {}
        pools["mov"] = ctx.enter_context(tc.tile_pool(name="mov", bufs=3))
        pools["psA"] = ctx.enter_context(tc.tile_pool(name="psA", bufs=2, space="PSUM"))
        pools["psB"] = ctx.enter_context(tc.tile_pool(name="psB", bufs=4, space="PSUM"))
        pools["xgu"] = ctx.enter_context(tc.tile_pool(name="xgu", bufs=2))
        pools["hfl"] = ctx.enter_context(tc.tile_pool(name="hfl", bufs=2))
        for nm, nb in (("g", 4), ("t1", 4), ("tc", 4)):
            pools[nm] = ctx.enter_context(tc.tile_pool(name=nm, bufs=nb))
        singles = ctx.enter_context(tc.tile_pool(name="singles", bufs=1))
        wih_sb = singles.tile([P, KT, G], BF16, tag="wih")
        whh_sb = singles.tile([P, KT, G], BF16, tag="whh")
        bias_sb = singles.tile([P, MT], FP32, tag="bias")
        xg_sb = [singles.tile([P, MT, RB], XG2, tag=f"xg{i}", name=f"xg{i}")
                 for i in range(2)]
        h_k = [singles.tile([P, Bc], BF16, tag=f"h{k}", name=f"h{k}")
               for k in range(KT)]
        cg_k = [singles.tile([P, 2, Bc], FP32, tag=f"cg{k}", name=f"cg{k}")
                for k in range(KT)]
        hint = (mybir.EngineType.PE, mybir.EngineType.DVE,
                mybir.EngineType.Activation, mybir.EngineType.SP)

        nc.sync.dma_start(out=wih_sb, in_=wih.rearrange("a p g -> p a g"))
        nc.sync.dma_start(out=whh_sb, in_=whh.rearrange("a p g -> p a g"))
        nc.sync.dma_start(out=bias_sb, in_=bias.rearrange("m p -> p m"))

        eng = nc.sync
        l_sv = _load_ctrl(nc, eng, ctrl, 0, 3)
        first_sv = _load_ctrl(nc, eng, ctrl, 1, 2 * (L - 1))
        last_sv = _load_ctrl(nc, eng, ctrl, 2, NB - 1 + 2 * (L - 1))
        pslot_sv = _load_ctrl(nc, eng, ctrl, 3, 3)
        is_l0 = 1 - s_not_equal(l_sv, 0)

        # pre-phase: xg for block 0 into slot 0 (gath garbage for l>0)
        _emit_phase_a2(nc, pools, wih_sb, bias_sb, xg_sb[0], gath[0], pslot_sv,
                       xT, 0, is_l0, RB, CH)

        for j in range(NJ):
            if not NO_CC:
                nc.gpsimd.collective_compute(
                    kind="AllGather", op=mybir.AluOpType.bypass,
                    replica_groups=[[0, 1, 2, 3], [4, 5, 6, 7]],
                    ins=[sendb[(j - 1) % 2]], outs=[gath[j % 2]],
                )
            is_first = 1 - s_not_equal(first_sv, j)
            for k in range(KT):
                nc.sync.dma_start(out=h_k[k], in_=h0[k], cond=is_first)
                nc.sync.dma_start(out=cg_k[k][:, 0, :], in_=c0[k], cond=is_first)
            nst = BLK if FAKE_STEPS < 0 else FAKE_STEPS
            if nst:
                _emit_steps2(nc, tc, pools, whh_sb, xg_sb[j % 2], h_k, cg_k,
                             sendb[j % 2], nst, U, hint)
            if not NO_PHA:
                # phase A for next block into the other slot
                l0_roff = min((j + 1) * RB, (NB - 1) * RB)
                _emit_phase_a2(nc, pools, wih_sb, bias_sb, xg_sb[(j + 1) % 2],
                               gath[j % 2], pslot_sv, xT, l0_roff, is_l0, RB, CH)
            is_last = 1 - s_not_equal(last_sv, j)
            for k in range(KT):
                nc.sync.dma_start(out=cout[k], in_=cg_k[k][:, 0, :], cond=is_last)
    nc.compile()
    return nc


def _prep_pipe2(x, h0, c0, w_ih, w_hh, b_ih, b_hh, Tl, BLK):
    Bc = BC2
    NB = Tl // BLK
    w_ih, w_hh = _perm_gates2(w_ih), _perm_gates2(w_hh)
    b = _perm_gates2((np.asarray(b_ih, np.float32)
                      + np.asarray(b_hh, np.float32))[..., None])[..., 0]
    wihT = np.ascontiguousarray(_bf16(w_ih).transpose(0, 2, 1).reshape(L, KT, P, G))
    whhT = np.ascontiguousarray(_bf16(w_hh).transpose(0, 2, 1).reshape(L, KT, P, G))
    ins = []
    for c in range(8):
        half, l = c // 4, c % 4
        bs = slice(half * Bc, (half + 1) * Bc)
        xc = np.asarray(x[bs, :Tl, :], np.float32)
        xT = np.ascontiguousarray(_bf16(xc).transpose(2, 1, 0).reshape(KT, P, Tl * Bc))
        h0T = np.ascontiguousarray(_bf16(h0[l, bs, :]).T.reshape(KT, P, Bc))
        c0T = np.ascontiguousarray(
            np.asarray(c0[l, bs, :], np.float32).T.reshape(KT, P, Bc))
        ctrl = np.array([[l, 2 * l, NB - 1 + 2 * l, (l + 3) % 4]], np.uint32)
        ins.append({"xT": xT, "wihT": wihT[l], "whhT": whhT[l],
                    "bias": b[l].reshape(MT, P), "h0T": h0T, "c0T": c0T,
                    "ctrl": ctrl})
    return ins


# ---------------------------------------------------------------------------

def _get_built(mode, Tl):
    key = (mode, Tl)
    if key not in _cache:
        if mode == "split":
            _cache[key] = _build_split(Tl)
        elif mode == "pipe2":
            BLK = int(os.environ.get("LSTM_BLK", "128"))
            _cache[key] = _build_pipe2(Tl, BLK)
        else:
            BLK = int(os.environ.get("LSTM_BLK", "256"))
            _cache[key] = _build_pipe(Tl, BLK)
    return _cache[key]


def kernel(x, h0, c0, w_ih, w_hh, b_ih, b_hh):
    Tl = min(T_OV, np.asarray(x).shape[1])
    nc = _get_built(MODE, Tl)
    if MODE == "split":
        ins = _prep_split(x, h0, c0, w_ih, w_hh, b_ih, b_hh, Tl)
    elif MODE == "pipe2":
        BLK = int(os.environ.get("LSTM_BLK", "128"))
        ins = _prep_pipe2(x, h0, c0, w_ih, w_hh, b_ih, b_hh, Tl, BLK)
    else:
        BLK = int(os.environ.get("LSTM_BLK", "256"))
        ins = _prep_pipe(x, h0, c0, w_ih, w_hh, b_ih, b_hh, Tl, BLK)
    res = run_bass_kernel_spmd(nc, ins, core_ids=list(range(8)))
    out = _post_split(res.results) if MODE == "split" else _post_pipe(res.results)
    return out



# revision 29
# speedup vs baseline: 4.5436x; 3.1136x over previous
# Trainium2 Bass kernel for a 4-layer LSTM (B=32, T=2048, I=H=512),
# output = final cell states c_n (4, 32, 512).
#
# Mode "pipe2" (default): 8 cores = 4 layers x 2 batch halves, wavefront
# pipeline over time-blocks of BLK steps with a d=2 iteration lag per layer
# hop so the per-block AllGather (h-sequence handoff to the next layer) and
# phase A (input projection xg = Wih.T @ h_prev + b, computed with N=512
# matmuls) both overlap the sequential recurrence of the previous block:
#   iter j: [CC_j gathers blocks produced in iter j-1 (runs during steps)]
#           [steps_j: BLK recurrence steps on xg prepared in iter j-1]
#           [phaseA_j: xg for the block received by CC_j -> other xg slot]
# xg lives in SBUF (bf16, double-buffered) - no DRAM round trip.
#
# Per step: gates psum accumulates over 4 k-input sweeps (sweep k reads only
# h k-tile k of the previous step), with gate tiles regrouped so each
# 128-dim output group kappa = (f,i,o,g)[kappa] owns one PSUM bank. The
# per-kappa nonlinearity chains (sigmoid/tanh -> c update -> h) then start
# while the PE is still sweeping other groups, and the next step's k=0
# sweep starts as soon as h[kappa=0] is ready. PSUM has_written is cleared
# bank-wide by start=True, so only the bank's first matmul of a step sets it.
# The recurrence is LDW-bound: 64 (fast-weight-load bf16 LDW + N=16 MM)
# pairs per step.
#
# Mode "pipe" (previous) and "split" kept for A/B reference.

import os
import numpy as np
import ml_dtypes

import concourse.bass as bass
import concourse.tile as tile
from concourse import bacc, mybir
from concourse.bass import ds
from concourse.bass_utils import run_bass_kernel_spmd
from concourse.expressions import smin, smax, s_not_equal

BF16 = mybir.dt.bfloat16
FP32 = mybir.dt.float32

# Problem constants (hardcoded per the contract)
B, T, I = 32, 2048, 512
H, L, G = 512, 4, 2048  # G = 4*H gates
KT = 4        # k tiles (512 / 128)
MT = 16       # m (gate) tiles (2048 / 128)
P = 128

MODE = os.environ.get("LSTM_MODE", "pipe2")
BC2 = 16      # per-core batch in pipe2 (half of B)
T_OV = int(os.environ.get("LSTM_T_OVERRIDE", "0")) or T  # dev-only override
U_STEPS = int(os.environ.get("LSTM_U", "16"))
NO_CC = bool(int(os.environ.get("LSTM_NO_CC", "0")))  # timing diagnostic only
STAGGER = bool(int(os.environ.get("LSTM_STAGGER", "0")))
FAKE_STEPS = int(os.environ.get("LSTM_FAKE_STEPS", "-1"))  # diagnostic only
NO_PHA = bool(int(os.environ.get("LSTM_NO_PHA", "0")))  # diagnostic only
XG_DT_ENV = os.environ.get("LSTM_XG16", "0")  # bf16 xg scratch (halves DMA)
XG2 = mybir.dt.float32 if os.environ.get("LSTM_XG32", "0") == "1" else mybir.dt.bfloat16
KINNER = bool(int(os.environ.get("LSTM_KINNER", "0")))
PSB = int(os.environ.get("LSTM_PSB", "4"))
IDMM = bool(int(os.environ.get("LSTM_IDMM", "0")))  # xg into psum via identity MM
SHG = bool(int(os.environ.get("LSTM_SHG", "0")))    # Shared addr space for gath
F8 = bool(int(os.environ.get("LSTM_F8", "0")))      # fp8 h-path (whh/wih/x/h)
HDT = mybir.dt.float8e4 if F8 else mybir.dt.bfloat16


def _hcast(a):
    return np.asarray(a, np.float32).astype(mybir.dt.np(HDT))

# gate-tile order in the fused weight layout: i,f,o,g so the three sigmoid
# regions are contiguous (2 ACT calls instead of 3)
GORDER = (0, 1, 3, 2)  # block g <-> o swap applied to (i,f,g,o) weight rows

_cache = {}


def _bf16(a):
    return np.asarray(a, np.float32).astype(ml_dtypes.bfloat16)


def _perm_gates(w):
    """Reorder gate blocks (i,f,g,o) -> (i,f,o,g) along dim 1 of (L, 4H, ...)."""
    w = np.asarray(w)
    blocks = w.reshape(w.shape[0], 4, H, *w.shape[2:])
    return np.ascontiguousarray(blocks[:, GORDER].reshape(w.shape))


# ---------------------------------------------------------------------------
# shared emitters
# ---------------------------------------------------------------------------

def _emit_phase_a(nc, pools, wih_sb, bias_sb, src_ap, src_roff, xg_dram, rows):
    """xg[g, r] = Wih.T @ inp + bias for `rows` rows starting at src_roff
    (ScalarValue or int offset into src_ap's last dim). Writes xg_dram
    (MT, 128, rows) fp32."""
    CH = 512
    nch = rows // CH
    for c in range(nch):
        inp = pools["mov"].tile([P, KT, CH], BF16, tag="mov")
        off = src_roff + c * CH if not isinstance(src_roff, int) else src_roff + c * CH
        nc.sync.dma_start(
            out=inp, in_=src_ap[:, :, ds(off, CH)].rearrange("a p c -> p a c")
        )
        for m in range(MT):
            ps = pools["psA"].tile([P, CH], FP32, tag="psA")
            for k in range(KT):
                nc.tensor.matmul(
                    ps,
                    lhsT=wih_sb[:, k, m * P:(m + 1) * P],
                    rhs=inp[:, k, :],
                    start=(k == 0),
                    stop=(k == KT - 1),
                )
            xs = pools["xgs"].tile([P, CH], xg_dram.dtype, tag="xgs")
            nc.vector.tensor_scalar_add(xs, ps, bias_sb[:, m:m + 1])
            nc.sync.dma_start(out=xg_dram[m, :, c * CH:(c + 1) * CH], in_=xs)


def _emit_steps(nc, tc, pools, whh_sb, xg_dram, h_sb, c_sb, hseq_ap, hseq_roff,
                nsteps, Bc, U, hint):
    """The sequential recurrence: nsteps LSTM steps. Reads xg_dram
    (MT,128,nsteps*Bc) fp32; h_sb (128,KT,Bc) bf16 and c_sb (128,KT,Bc) fp32
    updated in place; writes h rows into hseq_ap[:, :, hseq_roff + s]."""
    rows_per_iter = U * Bc

    with tc.For_i(0, nsteps * Bc, rows_per_iter, hint_engines=hint,
                  staggered_reset=STAGGER) as s:
        xg_u = pools["xgu"].tile([P, MT, rows_per_iter], xg_dram.dtype, tag="xgu")
        nc.sync.dma_start(
            out=xg_u,
            in_=xg_dram[:, :, ds(s, rows_per_iter)].rearrange("m p c -> p m c"),
        )
        hfl = pools["hfl"].tile([P, KT, rows_per_iter], HDT, tag="hfl")
        for u in range(U):
            ps = pools["psB"].tile([P, MT, Bc], FP32, tag="psB")
            for m in range(MT):
                for k in range(KT):
                    nc.tensor.matmul(
                        ps[:, m, :],
                        lhsT=whh_sb[:, k, m * P:(m + 1) * P],
                        rhs=h_sb[:, k, :],
                        start=(k == 0),
                        stop=(k == KT - 1),
                    )
            z = pools["z"].tile([P, MT, Bc], FP32, tag="z")
            nc.vector.tensor_add(out=z, in0=ps, in1=xg_u[:, :, u * Bc:(u + 1) * Bc])
            # weight rows are pre-permuted to (i, f, o, g) on the host
            gts = pools["g"].tile([P, MT, Bc], FP32, tag="g")
            nc.scalar.activation(gts[:, 0:12, :], z[:, 0:12, :],
                                 mybir.ActivationFunctionType.Sigmoid)
            nc.scalar.activation(gts[:, 12:16, :], z[:, 12:16, :],
                                 mybir.ActivationFunctionType.Tanh)
            t1 = pools["t1"].tile([P, KT, Bc], FP32, tag="t1")
            t2 = pools["t2"].tile([P, KT, Bc], FP32, tag="t2")
            nc.vector.tensor_mul(t1, gts[:, 4:8, :], c_sb)      # f * c
            nc.vector.tensor_mul(t2, gts[:, 0:4, :], gts[:, 12:16, :])  # i * g
            nc.vector.tensor_add(c_sb, t1, t2)
            tc_t = pools["tc"].tile([P, KT, Bc], FP32, tag="tc")
            nc.scalar.activation(tc_t, c_sb, mybir.ActivationFunctionType.Tanh)
            nc.vector.tensor_mul(h_sb, gts[:, 8:12, :], tc_t)  # o * tanh(c) -> bf16
            nc.vector.tensor_copy(out=hfl[:, :, u * Bc:(u + 1) * Bc], in_=h_sb)
        hout_off = hseq_roff + s if not isinstance(hseq_roff, int) else hseq_roff + s
        nc.sync.dma_start(
            out=hseq_ap[:, :, ds(hout_off, rows_per_iter)].rearrange("a p c -> p a c"),
            in_=hfl,
        )


def _make_pools(tc, ctx, Bc, U):
    pools = {}
    pools["mov"] = ctx.enter_context(tc.tile_pool(name="mov", bufs=3))
    pools["psA"] = ctx.enter_context(tc.tile_pool(name="psA", bufs=2, space="PSUM"))
    pools["xgs"] = ctx.enter_context(tc.tile_pool(name="xgs", bufs=3))
    pools["xgu"] = ctx.enter_context(tc.tile_pool(name="xgu", bufs=2))
    pools["hfl"] = ctx.enter_context(tc.tile_pool(name="hfl", bufs=2))
    pools["psB"] = ctx.enter_context(tc.tile_pool(name="psB", bufs=2, space="PSUM"))
    for nm in ("z", "g", "t1", "t2", "tc"):
        pools[nm] = ctx.enter_context(tc.tile_pool(name=nm, bufs=2))
    return pools


# ---------------------------------------------------------------------------
# mode "split": batch-parallel, all layers per core
# ---------------------------------------------------------------------------

def _build_split(Tl):
    Bc = B // 8  # 4
    U = U_STEPS
    RT = Tl * Bc
    nc = bacc.Bacc("TRN2", target_bir_lowering=False, debug=False, num_devices=8)
    xT = nc.dram_tensor("xT", [KT, P, RT], BF16, kind="ExternalInput").ap()
    wih = nc.dram_tensor("wihT", [L, KT, P, G], BF16, kind="ExternalInput").ap()
    whh = nc.dram_tensor("whhT", [L, KT, P, G], BF16, kind="ExternalInput").ap()
    bias = nc.dram_tensor("bias", [L, MT, P], FP32, kind="ExternalInput").ap()
    h0 = nc.dram_tensor("h0T", [L, KT, P, Bc], BF16, kind="ExternalInput").ap()
    c0 = nc.dram_tensor("c0T", [L, KT, P, Bc], FP32, kind="ExternalInput").ap()
    cout = nc.dram_tensor("cT", [L, KT, P, Bc], FP32, kind="ExternalOutput").ap()

    XG = BF16 if XG_DT_ENV == "1" else FP32
    xg_d = nc.dram_tensor("xg", [MT, P, RT], XG, kind="Internal").ap()
    hs_a = nc.dram_tensor("hseqA", [KT, P, RT], BF16, kind="Internal").ap()
    hs_b = nc.dram_tensor("hseqB", [KT, P, RT], BF16, kind="Internal").ap()

    from contextlib import ExitStack
    with tile.TileContext(nc) as tc, ExitStack() as ctx:
        pools = _make_pools(tc, ctx, Bc, U)
        singles = ctx.enter_context(tc.tile_pool(name="singles", bufs=1))
        wih_sb = singles.tile([P, KT, G], BF16, tag="wih")
        whh_sb = singles.tile([P, KT, G], BF16, tag="whh")
        bias_sb = singles.tile([P, MT], FP32, tag="bias")
        h_sb = singles.tile([P, KT, Bc], BF16, tag="h")
        c_sb = singles.tile([P, KT, Bc], FP32, tag="c")
        hint = (mybir.EngineType.PE, mybir.EngineType.DVE,
                mybir.EngineType.Activation, mybir.EngineType.SP)

        for l in range(L):
            nc.sync.dma_start(out=wih_sb, in_=wih[l].rearrange("a p g -> p a g"))
            nc.sync.dma_start(out=whh_sb, in_=whh[l].rearrange("a p g -> p a g"))
            nc.sync.dma_start(out=bias_sb, in_=bias[l].rearrange("m p -> p m"))
            nc.sync.dma_start(out=h_sb, in_=h0[l].rearrange("a p b -> p a b"))
            nc.sync.dma_start(out=c_sb, in_=c0[l].rearrange("a p b -> p a b"))
            src = xT if l == 0 else (hs_a if l % 2 == 1 else hs_b)
            dst = hs_a if l % 2 == 0 else hs_b
            _emit_phase_a(nc, pools, wih_sb, bias_sb, src, 0, xg_d, RT)
            _emit_steps(nc, tc, pools, whh_sb, xg_d, h_sb, c_sb, dst, 0,
                        Tl, Bc, U, hint)
            nc.sync.dma_start(out=cout[l].rearrange("a p b -> p a b"), in_=c_sb)
    nc.compile()
    return nc


def _prep_split(x, h0, c0, w_ih, w_hh, b_ih, b_hh, Tl):
    Bc = B // 8
    w_ih, w_hh = _perm_gates(w_ih), _perm_gates(w_hh)
    b_ih, b_hh = _perm_gates(b_ih[..., None])[..., 0], _perm_gates(b_hh[..., None])[..., 0]
    ins = []
    wihT = np.ascontiguousarray(
        _bf16(w_ih).transpose(0, 2, 1).reshape(L, KT, P, G))
    whhT = np.ascontiguousarray(
        _bf16(w_hh).transpose(0, 2, 1).reshape(L, KT, P, G))
    bias = np.ascontiguousarray(
        (np.asarray(b_ih, np.float32) + np.asarray(b_hh, np.float32))
        .reshape(L, MT, P))
    for c in range(8):
        bs = slice(c * Bc, (c + 1) * Bc)
        # xT[kt, p, t*Bc + b] = x[b, t, kt*128+p]
        xc = np.asarray(x[bs, :Tl, :], np.float32)  # (Bc, Tl, I)
        xT = np.ascontiguousarray(
            _bf16(xc).transpose(2, 1, 0).reshape(KT, P, Tl * Bc))
        h0T = np.ascontiguousarray(
            _bf16(h0[:, bs, :]).transpose(0, 2, 1).reshape(L, KT, P, Bc))
        c0T = np.ascontiguousarray(
            np.asarray(c0[:, bs, :], np.float32).transpose(0, 2, 1)
            .reshape(L, KT, P, Bc))
        ins.append({"xT": xT, "wihT": wihT, "whhT": whhT, "bias": bias,
                    "h0T": h0T, "c0T": c0T})
    return ins


def _post_split(results):
    Bc = B // 8
    out = np.zeros((L, B, H), np.float32)
    for c, r in enumerate(results):
        ct = r["cT"]  # (L, KT, P, Bc)
        out[:, c * Bc:(c + 1) * Bc, :] = ct.reshape(L, H, Bc).transpose(0, 2, 1)
    return out


# ---------------------------------------------------------------------------
# mode "pipe": layer pipeline x batch halves
# ---------------------------------------------------------------------------

def _build_pipe(Tl, BLK):
    Bc = B // 2  # 16
    U = U_STEPS
    NB = Tl // BLK
    RB = BLK * Bc          # rows per block
    RT = Tl * Bc
    LAG = L - 1
    nc = bacc.Bacc("TRN2", target_bir_lowering=False, debug=False, num_devices=8)
    xT = nc.dram_tensor("xT", [KT, P, RT], BF16, kind="ExternalInput").ap()
    wih = nc.dram_tensor("wihT", [KT, P, G], BF16, kind="ExternalInput").ap()
    whh = nc.dram_tensor("whhT", [KT, P, G], BF16, kind="ExternalInput").ap()
    bias = nc.dram_tensor("bias", [MT, P], FP32, kind="ExternalInput").ap()
    h0 = nc.dram_tensor("h0T", [KT, P, Bc], BF16, kind="ExternalInput").ap()
    c0 = nc.dram_tensor("c0T", [KT, P, Bc], FP32, kind="ExternalInput").ap()
    # ctrl scalars: [l, l*RB, prev_slot]
    ctrl = nc.dram_tensor("ctrl", [1, 4], mybir.dt.uint32, kind="ExternalInput").ap()
    cout = nc.dram_tensor("cT", [KT, P, Bc], FP32, kind="ExternalOutput").ap()

    XG = BF16 if XG_DT_ENV == "1" else FP32
    xg_d = nc.dram_tensor("xg", [MT, P, RB], XG, kind="Internal").ap()
    sendb = nc.dram_tensor("sendb", [KT, P, RB], BF16, kind="Internal").ap()
    gath = nc.dram_tensor("gath", [4, KT, P, RB], BF16, kind="Internal").ap()

    from contextlib import ExitStack
    with tile.TileContext(nc) as tc, ExitStack() as ctx:
        pools = _make_pools(tc, ctx, Bc, U)
        singles = ctx.enter_context(tc.tile_pool(name="singles", bufs=1))
        wih_sb = singles.tile([P, KT, G], BF16, tag="wih")
        whh_sb = singles.tile([P, KT, G], BF16, tag="whh")
        bias_sb = singles.tile([P, MT], FP32, tag="bias")
        h_sb = singles.tile([P, KT, Bc], BF16, tag="h")
        c_sb = singles.tile([P, KT, Bc], FP32, tag="c")
        hint = (mybir.EngineType.PE, mybir.EngineType.DVE,
                mybir.EngineType.Activation, mybir.EngineType.SP)

        nc.sync.dma_start(out=wih_sb, in_=wih.rearrange("a p g -> p a g"))
        nc.sync.dma_start(out=whh_sb, in_=whh.rearrange("a p g -> p a g"))
        nc.sync.dma_start(out=bias_sb, in_=bias.rearrange("m p -> p m"))

        eng = nc.sync
        l_sv = _load_ctrl(nc, eng, ctrl, 0, 3)
        lrb_sv = _load_ctrl(nc, eng, ctrl, 1, LAG * RB)
        pslot_sv = _load_ctrl(nc, eng, ctrl, 2, 3)

        for j in range(NB + LAG):
            # block index this core works on: clamp(j - l, 0, NB-1) * RB
            roff = smax(smin(j * RB - lrb_sv, (NB - 1) * RB), 0)
            # exchange h blocks (contents of sendb were written in iter j-1)
            if not NO_CC:
                nc.gpsimd.collective_compute(
                    kind="AllGather", op=mybir.AluOpType.bypass,
                    replica_groups=[[0, 1, 2, 3], [4, 5, 6, 7]],
                    ins=[sendb], outs=[gath],
                )
            # receive predecessor's block into my input sequence (l>0 only)
            nc.sync.dma_start(
                out=xT[:, :, ds(roff, RB)],
                in_=gath[ds(pslot_sv, 1), :, :, :].rearrange("o a p c -> (o a) p c"),
                cond=s_not_equal(l_sv, 0),
            )
            # state init on my first real block
            is_first = 1 - s_not_equal(l_sv, j)
            nc.sync.dma_start(out=h_sb, in_=h0.rearrange("a p b -> p a b"),
                              cond=is_first)
            nc.sync.dma_start(out=c_sb, in_=c0.rearrange("a p b -> p a b"),
                              cond=is_first)
            if not NO_PHA:
                _emit_phase_a(nc, pools, wih_sb, bias_sb, xT, roff, xg_d, RB)
            nst = BLK if FAKE_STEPS < 0 else FAKE_STEPS
            if nst:
                _emit_steps(nc, tc, pools, whh_sb, xg_d, h_sb, c_sb, sendb, 0,
                            nst, Bc, U, hint)
            # write final c on my last real block
            is_last = 1 - s_not_equal(l_sv, j - NB + 1)
            nc.sync.dma_start(out=cout.rearrange("a p b -> p a b"), in_=c_sb,
                              cond=is_last)
    nc.compile()
    return nc


def _load_ctrl(nc, eng, ctrl, idx, max_val):
    reg = eng.alloc_register(f"ctrl{idx}")
    eng.reg_load(reg, ctrl[0:1, idx:idx + 1])
    return eng.snap(reg, donate=True, min_val=0, max_val=max_val)


def _prep_pipe(x, h0, c0, w_ih, w_hh, b_ih, b_hh, Tl, BLK):
    Bc = B // 2
    w_ih, w_hh = _perm_gates(w_ih), _perm_gates(w_hh)
    b_ih, b_hh = _perm_gates(b_ih[..., None])[..., 0], _perm_gates(b_hh[..., None])[..., 0]
    RB = BLK * Bc
    bias_all = (np.asarray(b_ih, np.float32) + np.asarray(b_hh, np.float32))
    wihT = np.ascontiguousarray(_bf16(w_ih).transpose(0, 2, 1).reshape(L, KT, P, G))
    whhT = np.ascontiguousarray(_bf16(w_hh).transpose(0, 2, 1).reshape(L, KT, P, G))
    ins = []
    for c in range(8):
        half, l = c // 4, c % 4
        bs = slice(half * Bc, (half + 1) * Bc)
        xc = np.asarray(x[bs, :Tl, :], np.float32)
        xT = np.ascontiguousarray(_bf16(xc).transpose(2, 1, 0).reshape(KT, P, Tl * Bc))
        h0T = np.ascontiguousarray(_bf16(h0[l, bs, :]).T.reshape(KT, P, Bc))
        c0T = np.ascontiguousarray(
            np.asarray(c0[l, bs, :], np.float32).T.reshape(KT, P, Bc))
        ctrl = np.array([[l, l * RB, (l + 3) % 4, 0]], np.uint32)
        ins.append({"xT": xT, "wihT": wihT[l], "whhT": whhT[l],
                    "bias": bias_all[l].reshape(MT, P), "h0T": h0T, "c0T": c0T,
                    "ctrl": ctrl})
    return ins


def _post_pipe(results):
    Bc = B // 2
    out = np.zeros((L, B, H), np.float32)
    for c, r in enumerate(results):
        half, l = c // 4, c % 4
        ct = r["cT"]  # (KT, P, Bc)
        out[l, half * Bc:(half + 1) * Bc, :] = ct.reshape(H, Bc).T
    return out


# ---------------------------------------------------------------------------
# mode "pipe2": layer pipeline x batch halves, d=2 overlapped CC/phaseA,
# SBUF-resident xg, k-sweep step order with per-kappa-group chains.
# ---------------------------------------------------------------------------
# Gate-tile order m' = kappa*4 + gate with gate order (f, i, o, g); kappa is
# the 128-dim output h-group. Host permutes weight rows accordingly.

GORDER2 = (1, 0, 3, 2)  # per-group gate source blocks: f, i, o, g


def _perm_gates2(w):
    """Rows (4H) in blocks (i,f,g,o) -> m' order: for kappa, (f,i,o,g)[kappa]."""
    w = np.asarray(w)
    blocks = w.reshape(w.shape[0], 4, KT, P, *w.shape[2:])  # (L, gate, kappa, p, ...)
    out = blocks[:, GORDER2]  # (L, gate'(f,i,o,g), kappa, p, ...)
    out = out.transpose(0, 2, 1, *range(3, out.ndim))  # (L, kappa, gate', p, ...)
    return np.ascontiguousarray(out.reshape(w.shape))


def _emit_phase_a2(nc, pools, wih_sb, bias_sb, xg_dst, src_gath, pslot_sv, xT,
                   l0_roff, is_l0, rows, CH):
    """Compute xg[m', r] for `rows` rows into xg_dst (SBUF [P, MT, rows] bf16).
    Source: xT[:, :, l0_roff + ...] when is_l0 else gath[pslot]."""
    nch = rows // CH
    for c in range(nch):
        inp = pools["mov"].tile([P, KT, CH], HDT, tag="mov")
        nc.sync.dma_start(
            out=inp, in_=xT[:, :, ds(l0_roff + c * CH, CH)].rearrange("a p c -> p a c"),
            cond=is_l0,
        )
        nc.sync.dma_start(
            out=inp,
            in_=src_gath[ds(pslot_sv, 1), :, :, ds(c * CH, CH)]
            .rearrange("o a p c -> p (o a) c"),
            cond=1 - is_l0,
        )
        for m in range(MT):
            ps = pools["psA"].tile([P, CH], FP32, tag="psA")
            for k in range(KT):
                nc.tensor.matmul(
                    ps,
                    lhsT=wih_sb[:, k, m * P:(m + 1) * P],
                    rhs=inp[:, k, :],
                    start=(k == 0),
                    stop=(k == KT - 1),
                )
            nc.vector.tensor_scalar_add(
                xg_dst[:, m, c * CH:(c + 1) * CH], ps, bias_sb[:, m:m + 1])


def _emit_steps2(nc, tc, pools, whh_sb, xg_sb, h_k, cg_k, sendb, nsteps, U, hint,
                 ident_sb=None):
    """nsteps LSTM steps; xg_sb SBUF [P, MT, nsteps*Bc]; h_k[kt] [P,Bc] bf16
    persistent; cg_k[kt] [P,2,Bc] fp32 persistent (c in [:,0,:]); h rows
    written to sendb DRAM [KT, P, nsteps*Bc]."""
    Bc = BC2
    rows_per_iter = U * Bc
    with tc.For_i(0, nsteps * Bc, rows_per_iter, hint_engines=hint,
                  staggered_reset=bool(STAGGER)) as s:
        xg_u = pools["xgu"].tile([P, MT, rows_per_iter], XG2, tag="xgu")
        nc.sync.dma_start(out=xg_u, in_=xg_sb[:, :, ds(s, rows_per_iter)])
        hfl = pools["hfl"].tile([P, KT, rows_per_iter], HDT, tag="hfl")
        for u in range(U):
            ps_g = [pools["psB"].tile([P, 4, Bc], FP32, tag="ps", name=f"ps{i}")
                    for i in range(4)]
            use_id = ident_sb is not None
            if use_id:
                # preload psum with xg via identity matmul (no h dependency;
                # fills the h-wait stall and drops the DVE add from chains)
                for kap in range(4):
                    nc.tensor.matmul(
                        ps_g[kap],
                        lhsT=ident_sb,
                        rhs=xg_u[:, kap * 4:(kap + 1) * 4, u * Bc:(u + 1) * Bc],
                        start=True,
                        stop=False,
                    )
            # k-input-ordered sweeps; within a sweep, m' ordered by kappa
            def rhs_k(k):
                return h_k[k] if u == 0 else hfl[:, k, (u - 1) * Bc:u * Bc]
            if KINNER:
                order = [(k, kap, g) for kap in range(4) for g in range(4)
                         for k in range(KT)]
            else:
                order = [(k, kap, g) for k in range(KT) for kap in range(4)
                         for g in range(4)]
            for (k, kap, g) in order:
                mp = kap * 4 + g
                # start=True clears has_written for the WHOLE bank; with
                # k-outer order only the bank's first matmul may set it.
                st = False if use_id else ((k == 0) if KINNER
                                           else (k == 0 and g == 0))
                nc.tensor.matmul(
                    ps_g[kap][:, g, :],
                    lhsT=whh_sb[:, k, mp * P:(mp + 1) * P],
                    rhs=rhs_k(k),
                    start=st,
                    stop=(k == KT - 1),
                )
            for kap in range(4):
                ps = ps_g[kap]
                if not use_id:
                    # z = psum + xg (in place, PSUM)
                    nc.vector.tensor_add(
                        out=ps, in0=ps,
                        in1=xg_u[:, kap * 4:(kap + 1) * 4, u * Bc:(u + 1) * Bc])
                gts = pools["g"].tile([P, 3, Bc], FP32, tag="g")
                nc.scalar.activation(gts, ps[:, 0:3, :],
                                     mybir.ActivationFunctionType.Sigmoid)
                nc.scalar.activation(cg_k[kap][:, 1:2, :], ps[:, 3:4, :],
                                     mybir.ActivationFunctionType.Tanh)
                t12 = pools["t1"].tile([P, 2, Bc], FP32, tag="t1")
                nc.vector.tensor_mul(t12, gts[:, 0:2, :], cg_k[kap])  # f*c, i*tg
                nc.vector.tensor_add(cg_k[kap][:, 0:1, :],
                                     t12[:, 0:1, :], t12[:, 1:2, :])
                tcx = pools["tc"].tile([P, 1, Bc], FP32, tag="tc")
                nc.scalar.activation(tcx, cg_k[kap][:, 0:1, :],
                                     mybir.ActivationFunctionType.Tanh)
                nc.vector.tensor_mul(hfl[:, kap, u * Bc:(u + 1) * Bc],
                                     gts[:, 2, :], tcx[:, 0, :])
            if u == U - 1:
                for kap in range(4):
                    nc.scalar.copy(h_k[kap], hfl[:, kap, u * Bc:(u + 1) * Bc])
        nc.sync.dma_start(
            out=sendb[:, :, ds(s, rows_per_iter)].rearrange("a p c -> p a c"),
            in_=hfl,
        )


def _build_pipe2(Tl, BLK):
    Bc = BC2
    U = U_STEPS
    NB = Tl // BLK
    RB = BLK * Bc
    RT = Tl * Bc
    NJ = NB + 2 * (L - 1)
    CH = min(512, RB)
    nc = bacc.Bacc("TRN2", target_bir_lowering=False, debug=False, num_devices=8)
    xT = nc.dram_tensor("xT", [KT, P, RT], HDT, kind="ExternalInput").ap()
    wih = nc.dram_tensor("wihT", [KT, P, G], HDT, kind="ExternalInput").ap()
    whh = nc.dram_tensor("whhT", [KT, P, G], HDT, kind="ExternalInput").ap()
    bias = nc.dram_tensor("bias", [MT, P], FP32, kind="ExternalInput").ap()
    h0 = nc.dram_tensor("h0T", [KT, P, Bc], HDT, kind="ExternalInput").ap()
    c0 = nc.dram_tensor("c0T", [KT, P, Bc], FP32, kind="ExternalInput").ap()
    # ctrl scalars: [l, first_iter, last_iter, pslot]
    ctrl = nc.dram_tensor("ctrl", [1, 4], mybir.dt.uint32, kind="ExternalInput").ap()
    if IDMM:
        ident = nc.dram_tensor("ident", [P, P], BF16, kind="ExternalInput").ap()
    cout = nc.dram_tensor("cT", [KT, P, Bc], FP32, kind="ExternalOutput").ap()

    gspace = "Shared" if SHG else "Local"
    sendb = [nc.dram_tensor(f"sendb{i}", [KT, P, RB], HDT, kind="Internal").ap()
             for i in range(2)]
    gath = [nc.dram_tensor(f"gath{i}", [4, KT, P, RB], HDT, kind="Internal",
                           addr_space=gspace).ap()
            for i in range(2)]

    from contextlib import ExitStack
    with tile.TileContext(nc) as tc, ExitStack() as ctx:
        pools = {}
        pools["mov"] = ctx.enter_context(tc.tile_pool(name="mov", bufs=3))
        pools["psA"] = ctx.enter_context(tc.tile_pool(name="psA", bufs=2, space="PSUM"))
        pools["psB"] = ctx.enter_context(tc.tile_pool(name="psB", bufs=PSB, space="PSUM"))
        pools["xgu"] = ctx.enter_context(tc.tile_pool(name="xgu", bufs=2))
        pools["hfl"] = ctx.enter_context(tc.tile_pool(name="hfl", bufs=2))
        for nm, nb in (("g", 4), ("t1", 4), ("tc", 4)):
            pools[nm] = ctx.enter_context(tc.tile_pool(name=nm, bufs=nb))
        singles = ctx.enter_context(tc.tile_pool(name="singles", bufs=1))
        wih_sb = singles.tile([P, KT, G], HDT, tag="wih")
        whh_sb = singles.tile([P, KT, G], HDT, tag="whh")
        bias_sb = singles.tile([P, MT], FP32, tag="bias")
        xg_sb = [singles.tile([P, MT, RB], XG2, tag=f"xg{i}", name=f"xg{i}")
                 for i in range(2)]
        h_k = [singles.tile([P, Bc], HDT, tag=f"h{k}", name=f"h{k}")
               for k in range(KT)]
        cg_k = [singles.tile([P, 2, Bc], FP32, tag=f"cg{k}", name=f"cg{k}")
                for k in range(KT)]
        hint = (mybir.EngineType.PE, mybir.EngineType.DVE,
                mybir.EngineType.Activation, mybir.EngineType.SP)

        nc.sync.dma_start(out=wih_sb, in_=wih.rearrange("a p g -> p a g"))
        nc.sync.dma_start(out=whh_sb, in_=whh.rearrange("a p g -> p a g"))
        nc.sync.dma_start(out=bias_sb, in_=bias.rearrange("m p -> p m"))

        ident_sb = None
        if IDMM:
            ident_sb = singles.tile([P, P], BF16, tag="ident", name="ident_sb")
            nc.sync.dma_start(out=ident_sb, in_=ident)

        eng = nc.sync
        l_sv = _load_ctrl(nc, eng, ctrl, 0, 3)
        first_sv = _load_ctrl(nc, eng, ctrl, 1, 2 * (L - 1))
        last_sv = _load_ctrl(nc, eng, ctrl, 2, NB - 1 + 2 * (L - 1))
        pslot_sv = _load_ctrl(nc, eng, ctrl, 3, 3)
        is_l0 = 1 - s_not_equal(l_sv, 0)

        # pre-phase: xg for block 0 into slot 0 (gath garbage for l>0)
        _emit_phase_a2(nc, pools, wih_sb, bias_sb, xg_sb[0], gath[0], pslot_sv,
                       xT, 0, is_l0, RB, CH)

        for j in range(NJ):
            if not NO_CC:
                nc.gpsimd.collective_compute(
                    kind="AllGather", op=mybir.AluOpType.bypass,
                    replica_groups=[[0, 1, 2, 3], [4, 5, 6, 7]],
                    ins=[sendb[(j - 1) % 2]], outs=[gath[j % 2]],
                )
            is_first = 1 - s_not_equal(first_sv, j)
            for k in range(KT):
                nc.sync.dma_start(out=h_k[k], in_=h0[k], cond=is_first)
                nc.sync.dma_start(out=cg_k[k][:, 0, :], in_=c0[k], cond=is_first)
            nst = BLK if FAKE_STEPS < 0 else FAKE_STEPS
            if nst:
                _emit_steps2(nc, tc, pools, whh_sb, xg_sb[j % 2], h_k, cg_k,
                             sendb[j % 2], nst, U, hint, ident_sb=ident_sb)
            if not NO_PHA:
                # phase A for next block into the other slot
                l0_roff = min((j + 1) * RB, (NB - 1) * RB)
                _emit_phase_a2(nc, pools, wih_sb, bias_sb, xg_sb[(j + 1) % 2],
                               gath[j % 2], pslot_sv, xT, l0_roff, is_l0, RB, CH)
            is_last = 1 - s_not_equal(last_sv, j)
            for k in range(KT):
                nc.sync.dma_start(out=cout[k], in_=cg_k[k][:, 0, :], cond=is_last)
    nc.compile()
    return nc


def _prep_pipe2(x, h0, c0, w_ih, w_hh, b_ih, b_hh, Tl, BLK):
    Bc = BC2
    NB = Tl // BLK
    w_ih, w_hh = _perm_gates2(w_ih), _perm_gates2(w_hh)
    b = _perm_gates2((np.asarray(b_ih, np.float32)
                      + np.asarray(b_hh, np.float32))[..., None])[..., 0]
    wihT = np.ascontiguousarray(_hcast(w_ih).transpose(0, 2, 1).reshape(L, KT, P, G))
    whhT = np.ascontiguousarray(_hcast(w_hh).transpose(0, 2, 1).reshape(L, KT, P, G))
    ins = []
    for c in range(8):
        half, l = c // 4, c % 4
        bs = slice(half * Bc, (half + 1) * Bc)
        xc = np.asarray(x[bs, :Tl, :], np.float32)
        xT = np.ascontiguousarray(_hcast(xc).transpose(2, 1, 0).reshape(KT, P, Tl * Bc))
        h0T = np.ascontiguousarray(_hcast(h0[l, bs, :]).T.reshape(KT, P, Bc))
        c0T = np.ascontiguousarray(
            np.asarray(c0[l, bs, :], np.float32).T.reshape(KT, P, Bc))
        ctrl = np.array([[l, 2 * l, NB - 1 + 2 * l, (l + 3) % 4]], np.uint32)
        m = {"xT": xT, "wihT": wihT[l], "whhT": whhT[l],
             "bias": b[l].reshape(MT, P), "h0T": h0T, "c0T": c0T,
             "ctrl": ctrl}
        if IDMM:
            m["ident"] = np.ascontiguousarray(
                np.eye(P, dtype=np.float32).astype(ml_dtypes.bfloat16))
        ins.append(m)
    return ins


# ---------------------------------------------------------------------------

def _get_built(mode, Tl):
    key = (mode, Tl)
    if key not in _cache:
        if mode == "split":
            _cache[key] = _build_split(Tl)
        elif mode == "pipe2":
            BLK = int(os.environ.get("LSTM_BLK", "32"))
            _cache[key] = _build_pipe2(Tl, BLK)
        else:
            BLK = int(os.environ.get("LSTM_BLK", "256"))
            _cache[key] = _build_pipe(Tl, BLK)
    return _cache[key]


def kernel(x, h0, c0, w_ih, w_hh, b_ih, b_hh):
    Tl = min(T_OV, np.asarray(x).shape[1])
    nc = _get_built(MODE, Tl)
    if MODE == "split":
        ins = _prep_split(x, h0, c0, w_ih, w_hh, b_ih, b_hh, Tl)
    elif MODE == "pipe2":
        BLK = int(os.environ.get("LSTM_BLK", "32"))
        ins = _prep_pipe2(x, h0, c0, w_ih, w_hh, b_ih, b_hh, Tl, BLK)
    else:
        BLK = int(os.environ.get("LSTM_BLK", "256"))
        ins = _prep_pipe(x, h0, c0, w_ih, w_hh, b_ih, b_hh, Tl, BLK)
    res = run_bass_kernel_spmd(nc, ins, core_ids=list(range(8)))
    out = _post_split(res.results) if MODE == "split" else _post_pipe(res.results)
    return out

